# revision 1
# baseline (speedup 1.0000x reference)
"""Trainium2 Bass kernel for nn_Fractal1D (soft fractal / smoothed decision-tree descent).

Reference computation (per point x, N=131072 points, M=128 nodes, depth 10):
    split = sigmoid(4*p - 2); values = tile(3*v + 1, 4)
    w0 = e_0;  lo=0, hi=1
    repeat 10x:
        s  = lo + (w @ split) * (hi - lo)
        t  = sigmoid((x - s) / 0.1)
        w  = (1-t) * (w @ L) + t * (w @ R)
        lo, hi = (1-t)*lo + t*s, (1-t)*s + t*hi
    out = w @ values

Kernel strategy (data-parallel over 8 cores, 16384 points/core):
  * w^T resident in SBUF as [128 nodes, 16384 points]; points processed in
    32 chunks of 512 (one PSUM bank of fp32 per matmul).
  * Matmuls run as float32r (TF32-like, 1 col/cycle; plain fp32 is 4 col/cycle).
    The BIR verifier requires every producer of an f32r matmul operand to
    round on write, so all matmul-feeding tiles are declared float32r and
    DVE/ACT reads of them bitcast back to fp32 (bit-identical view).
  * Per depth:
      - "matvec"  sdot = split^T w  : 16 accumulating matmuls with one-hot-placed
        split columns (lhsT = split x e_i) stack 16 chunks into one [16,512] PSUM tile.
      - scalar state updates run 16-chunks-wide on partitions 0..15 using the
        substitution xml = x - lo, d = hi - lo:
            g    = sdot * d
            t    = sigmoid(10*(xml - g))
            xml' = xml - t*g
            d'   = g - 2*t*g + t*d
      - "broadcast" matmul (lhsT = one-hot E_i, rhs = t tile) replicates chunk i's
        t row across 128 partitions into PSUM.
      - v = w_chunk * t_bcast (DVE), then PSUM-accumulated blend:
            w' = L^T w_chunk + (R-L)^T v
      - w' copied PSUM->SBUF on the scalar engine (rounds to f32r).
  * Final: 16 accumulating matmuls with values x e_i weights -> [16,512], DMA out.
"""

from contextlib import ExitStack

import numpy as np

import concourse.bacc as bacc
import concourse.bass as bass
import concourse.tile as tile
from concourse import mybir
from concourse.bass_utils import run_bass_kernel_spmd

F32 = mybir.dt.float32
U32 = mybir.dt.uint32
ONE_F32_BITS = 0x3F800000
F32R = mybir.dt.float32r
BF16 = mybir.dt.bfloat16
AOP = mybir.AluOpType
AFT = mybir.ActivationFunctionType

N_TOTAL = 131072
NCORES = 8
NPTS = N_TOTAL // NCORES      # 16384 points per core
F = 512                       # points per chunk (one PSUM bank of fp32)
M = 128                       # fractal nodes
NCH = 32                      # chunks (= partitions used for batched row math)
DEPTH = 10
INV_SMOOTH = 10.0             # 1 / smoothing_width


def f32(ap):
    """View an f32r AP as plain fp32 for DVE/ACT reads (bit-identical)."""
    return ap.bitcast(F32)


def _emit(nc, bench_reps=1):
    x_in = nc.declare_dram_parameter("x", [NPTS], F32, isOutput=False)
    spp_in = nc.declare_dram_parameter("spp", [M], F32, isOutput=False)
    vp_in = nc.declare_dram_parameter("vp", [32], F32, isOutput=False)
    l_in = nc.declare_dram_parameter("lmat", [M, M], F32, isOutput=False)
    r_in = nc.declare_dram_parameter("rmat", [M, M], F32, isOutput=False)
    y_out = nc.declare_dram_parameter("y", [NPTS], F32, isOutput=True)

    G = NCH // 2  # 16 chunks per phase group

    with tile.TileContext(nc) as tc, ExitStack() as ctx:
        sing = ctx.enter_context(tc.tile_pool(name="sing", bufs=1))
        scratch = ctx.enter_context(tc.tile_pool(name="scratch", bufs=2))
        tpool = ctx.enter_context(tc.tile_pool(name="tpool", bufs=4))
        vpool = ctx.enter_context(tc.tile_pool(name="vpool", bufs=3))
        ps_sdot = ctx.enter_context(tc.tile_pool(name="ps_sdot", bufs=3, space="PSUM"))
        ps_t = ctx.enter_context(tc.tile_pool(name="ps_t", bufs=2, space="PSUM"))
        ps_w = ctx.enter_context(tc.tile_pool(name="ps_w", bufs=3, space="PSUM"))

        # ---- constants / parameter transforms ----
        l_sb = sing.tile([M, M], F32, tag="l_sb")
        r_sb = sing.tile([M, M], F32, tag="r_sb")
        nc.sync.dma_start(out=l_sb, in_=l_in[:, :])
        nc.sync.dma_start(out=r_sb, in_=r_in[:, :])
        l_r = sing.tile([M, M], F32R, tag="l_r")
        nc.scalar.copy(l_r, l_sb)
        rml = sing.tile([M, M], F32R, tag="rml")
        nc.vector.tensor_sub(rml, r_sb, l_sb)

        spp_sb = sing.tile([M, 1], F32, tag="spp_sb")
        nc.sync.dma_start(out=spp_sb, in_=spp_in[:].rearrange("(p f) -> p f", f=1))
        spp_pre = sing.tile([M, 1], F32, tag="spp_pre")
        nc.vector.tensor_scalar(spp_pre, spp_sb, 4.0, -2.0, op0=AOP.mult, op1=AOP.add)
        split_sb = sing.tile([M, 1], F32, tag="split_sb")
        nc.scalar.activation(split_sb, spp_pre, AFT.Sigmoid)

        # splitE[k, i*16+m] = split[k] * (m == i): one-hot placed split columns
        splitE = sing.tile([M, G * G], F32R, tag="splitE")
        nc.vector.memset(splitE.bitcast(U32), 0)
        for i in range(G):
            nc.vector.tensor_copy(splitE[:, i * G + i : i * G + i + 1], split_sb)

        # Esel[k, i*128+m] = (k == i) for k,i in [0,16): select+broadcast
        # weights. Engines address SBUF only at partition starts {0,32,64,96},
        # so build via iota compares; round 0 on DVE gates PE start, round 1
        # on the otherwise-idle GPSIMD engine.
        CHW = 1024
        NRND = G * M // CHW
        esel_t = [
            sing.tile([G, CHW], F32R, tag=f"esel{r}", name=f"esel{r}")
            for r in range(NRND)
        ]

        def esel_slice(i):
            r = i * M // CHW
            off = i * M - r * CHW
            return esel_t[r][:, off : off + M]

        with tc.tile_pool(name="setup", bufs=1) as setup:
            for rnd in range(NRND):
                eng = nc.vector if rnd == 0 else nc.gpsimd
                iot = setup.tile([G, CHW], mybir.dt.int16, tag="iot")
                nc.gpsimd.iota(
                    iot, pattern=[[1, CHW]], base=rnd * CHW, channel_multiplier=-M
                )
                c_ge = setup.tile([G, CHW], F32, tag="c_ge")
                eng.tensor_scalar(c_ge, iot, 0, None, op0=AOP.is_ge)
                c_le = setup.tile([G, CHW], F32, tag="c_le")
                eng.tensor_scalar(c_le, iot, M - 1, None, op0=AOP.is_le)
                eng.tensor_mul(esel_t[rnd], c_ge, c_le)

        # values[128] = 3*tile(vp,4) + 1, then one-hot placed like splitE
        v128 = sing.tile([M, 1], F32, tag="v128")
        vp_ap = vp_in[:]
        vp_bcast = bass.AP(tensor=vp_ap.tensor, offset=vp_ap.offset, ap=[[0, 4], [1, 32]])
        nc.sync.dma_start(out=v128, in_=vp_bcast)
        nc.vector.tensor_scalar(v128, v128, 3.0, 1.0, op0=AOP.mult, op1=AOP.add)
        valE = sing.tile([M, G * G], F32R, tag="valE")
        nc.vector.memset(valE.bitcast(U32), 0)
        for i in range(G):
            nc.vector.tensor_copy(valE[:, i * G + i : i * G + i + 1], v128)

        # ---- depth-0 specialization constants ----
        # w_0 = e_0, so w_1 = (1-t0) (x) L[0,:] + t0 (x) R[0,:]; per chunk
        # that is one fused op: w1 = t_b * (R-L)[0,:] + L0_rep.
        l0col = sing.tile([M, 1], F32, tag="l0col")
        nc.sync.dma_start(out=l0col, in_=l_in[0, :].rearrange("(p f) -> p f", f=1))
        r0col = sing.tile([M, 1], F32, tag="r0col")
        nc.sync.dma_start(out=r0col, in_=r_in[0, :].rearrange("(p f) -> p f", f=1))
        rml0 = sing.tile([M, 1], F32, tag="rml0")
        nc.vector.tensor_sub(rml0, r0col, l0col)

        # split[0] replicated over the row-math partitions, as sigmoid bias
        spp0 = sing.tile([G, 1], F32, tag="spp0")
        spp_ap = spp_in[:]
        spp0_bcast = bass.AP(tensor=spp_ap.tensor, offset=spp_ap.offset, ap=[[0, G], [1, 1]])
        nc.sync.dma_start(out=spp0, in_=spp0_bcast)
        sp0 = sing.tile([G, 1], F32, tag="sp0")
        nc.vector.tensor_scalar(sp0, spp0, 4.0, -2.0, op0=AOP.mult, op1=AOP.add)
        nc.scalar.activation(sp0, sp0, AFT.Sigmoid)
        b0 = sing.tile([G, 1], F32, tag="b0")
        nc.vector.tensor_scalar_mul(b0, sp0, -INV_SMOOTH)

        # ---- resident state (per group) ----
        w_bufs = [
            sing.tile([M, NPTS], F32R, tag="w_ping", name="w_ping"),
            sing.tile([M, NPTS], F32R, tag="w_pong", name="w_pong"),
        ]
        half = G * F
        xml = [
            sing.tile([G, F], F32, tag="xml0", name="xml0"),
            sing.tile([G, F], F32, tag="xml1", name="xml1"),
        ]
        dd = [
            sing.tile([G, F], F32, tag="dd0", name="dd0"),
            sing.tile([G, F], F32, tag="dd1", name="dd1"),
        ]

        MV_LAG = 3

        def body():
         for g in range(2):
            nc.sync.dma_start(
                out=xml[g],
                in_=x_in[g * half : (g + 1) * half].rearrange("(p f) -> p f", f=F),
            )

         # ---- depth 0: closed form state, w1 via fused outer-product blend ----
         ttile = [None, None]
         for g in range(2):
             t0 = tpool.tile([G, F], F32R, tag="t", name=f"t0g{g}")
             nc.scalar.activation(t0, xml[g], AFT.Sigmoid, bias=b0, scale=INV_SMOOTH)
             tgt = scratch.tile([G, F], F32, tag="tgt")
             nc.gpsimd.tensor_scalar_mul(tgt, f32(t0), sp0)
             nc.gpsimd.tensor_sub(xml[g], xml[g], tgt)
             ee = scratch.tile([G, F], F32, tag="ee")
             nc.gpsimd.tensor_scalar_mul(ee, tgt, -2.0)
             nc.gpsimd.tensor_add(ee, ee, f32(t0))
             nc.gpsimd.tensor_scalar(dd[g], ee, sp0, None, op0=AOP.add)
             ttile[g] = t0

         # Phase schedule: phases k = 0..2*DEPTH-1 map to (depth, group).
         # Phase k runs group k%2 at depth k//2. The matvec feeding phase k+2
         # is accumulated (lagged) inside phase k's chunk loop; the row math
         # for phase k+1 is emitted before phase k's chunks so its short
         # DVE->ACT chain hides under phase k's PE work.
         sdot_ps = [None] * (2 * DEPTH + 2)

         def row_math(k):
             """State update for phase k (depth k//2 >= 1, group k%2)."""
             g = k % 2
             sdot = sdot_ps[k]
             gt = scratch.tile([G, F], F32, tag="gt", name=f"gt{k}")
             nc.vector.tensor_mul(gt, sdot, dd[g])
             xms = scratch.tile([G, F], F32, tag="xms", name=f"xms{k}")
             nc.vector.tensor_sub(xms, xml[g], gt)
             tg_t = tpool.tile([G, F], F32R, tag="t", name=f"t{k}")
             nc.scalar.activation(tg_t, xms, AFT.Sigmoid, scale=INV_SMOOTH)
             tgt = scratch.tile([G, F], F32, tag="tgt", name=f"tgt{k}")
             nc.gpsimd.tensor_mul(tgt, f32(tg_t), gt)
             nc.gpsimd.tensor_sub(xml[g], xml[g], tgt)
             ee = scratch.tile([G, F], F32, tag="ee", name=f"ee{k}")
             nc.gpsimd.tensor_scalar_mul(ee, tgt, -2.0)
             nc.gpsimd.tensor_add(ee, ee, gt)
             td = scratch.tile([G, F], F32, tag="td", name=f"td{k}")
             nc.gpsimd.tensor_mul(td, f32(tg_t), dd[g])
             nc.gpsimd.tensor_add(dd[g], td, ee)
             ttile[g] = tg_t

         for k in range(2 * DEPTH):
             dep, g = k // 2, k % 2
             w_cur = w_bufs[dep % 2]
             w_new = w_bufs[(dep + 1) % 2]
             if 2 <= k + 1 < 2 * DEPTH:
                 row_math(k + 1)
             tg_t = ttile[g]
             sdot_next = ps_sdot.tile([G, F], F32, tag="sdot", name=f"sdot{k + 2}")
             sdot_ps[k + 2] = sdot_next
             mv_lhs = valE if k + 2 >= 2 * DEPTH else splitE
             mv_q = []
             for i in range(G):
                 ci = g * G + i
                 wnsl = w_new[:, ci * F : (ci + 1) * F]
                 tb = ps_t.tile([M, F], F32, tag="tb")
                 nc.tensor.matmul(
                     tb, lhsT=esel_slice(i), rhs=tg_t, start=True, stop=True
                 )
                 if dep == 0:
                     # w1 = t_b * (R-L)[0,:] + L[0,:], straight to SBUF
                     nc.vector.tensor_scalar(
                         wnsl, tb, rml0, l0col, op0=AOP.mult, op1=AOP.add
                     )
                 else:
                     wsl = w_cur[:, ci * F : (ci + 1) * F]
                     wn = ps_w.tile([M, F], F32, tag="wn")
                     nc.tensor.matmul(wn, lhsT=l_r, rhs=wsl, start=True, stop=False)
                     vv = vpool.tile([M, F], F32R, tag="vv")
                     nc.vector.tensor_mul(vv, f32(wsl), tb)
                     nc.tensor.matmul(wn, lhsT=rml, rhs=vv, start=False, stop=True)
                     nc.scalar.copy(wnsl, wn)
                 mv_q.append((i, wnsl))
                 if len(mv_q) > MV_LAG:
                     j, src_ap = mv_q.pop(0)
                     nc.tensor.matmul(
                         sdot_next,
                         lhsT=mv_lhs[:, j * G : (j + 1) * G],
                         rhs=src_ap,
                         start=(j == 0),
                         stop=False,
                     )
             for j, src_ap in mv_q:
                 nc.tensor.matmul(
                     sdot_next,
                     lhsT=mv_lhs[:, j * G : (j + 1) * G],
                     rhs=src_ap,
                     start=(j == 0),
                     stop=(j == G - 1),
                 )
             mv_q.clear()

         # ---- output: y accumulated per group in the final sdot tiles ----
         for g in range(2):
             ysb = scratch.tile([G, F], F32, tag="ysb", bufs=1, name=f"ysb{g}")
             nc.scalar.copy(ysb, sdot_ps[2 * DEPTH + g])
             nc.sync.dma_start(
                 out=y_out[g * half : (g + 1) * half].rearrange("(p f) -> p f", f=F),
                 in_=ysb,
             )

        if bench_reps > 1:
            with tc.For_i(0, bench_reps, 1):
                body()
        else:
            body()

    return nc


_CACHE = {}


def build_bench(reps):
    """Fresh module with the whole computation repeated `reps` times on-device."""
    nc = bacc.Bacc("TRN2", target_bir_lowering=False)
    _emit(nc, bench_reps=reps)
    nc.compile()
    return nc


def build_bass(compiled=True):
    """Build (and by default finalize) the Bacc module.

    compiled=False returns the pre-compile module for CoreSim runs.
    """
    if "nc" not in _CACHE:
        nc = bacc.Bacc("TRN2", target_bir_lowering=False)
        _emit(nc)
        _CACHE["nc"] = nc
    nc = _CACHE["nc"]
    if compiled and not _CACHE.get("compiled"):
        nc.compile()
        _CACHE["compiled"] = True
    return nc


def make_in_maps(x, split_points_param, values_param, left_matrix, right_matrix):
    x = np.ascontiguousarray(x, dtype=np.float32)
    shards = x.reshape(NCORES, NPTS)
    common = {
        "spp": np.ascontiguousarray(split_points_param, dtype=np.float32),
        "vp": np.ascontiguousarray(values_param, dtype=np.float32),
        "lmat": np.ascontiguousarray(left_matrix, dtype=np.float32),
        "rmat": np.ascontiguousarray(right_matrix, dtype=np.float32),
    }
    return [{"x": shards[i], **common} for i in range(NCORES)]


def kernel(x, split_points_param, values_param, left_matrix, right_matrix, max_depth):
    assert int(max_depth) == DEPTH
    nc = build_bass()
    in_maps = make_in_maps(
        x, split_points_param, values_param, left_matrix, right_matrix
    )
    res = run_bass_kernel_spmd(nc, in_maps, list(range(NCORES)))
    out = np.concatenate([res.results[i]["y"] for i in range(NCORES)])
    return out.astype(np.float32)



# revision 9
# speedup vs baseline: 1.0759x; 1.0759x over previous
"""Trainium2 Bass kernel for nn_Fractal1D (soft fractal / smoothed decision-tree descent).

Reference computation (per point x, N=131072 points, M=128 nodes, depth 10):
    split = sigmoid(4*p - 2); values = tile(3*v + 1, 4)
    w0 = e_0;  lo=0, hi=1
    repeat 10x:
        s  = lo + (w @ split) * (hi - lo)
        t  = sigmoid((x - s) / 0.1)
        w  = (1-t) * (w @ L) + t * (w @ R)
        lo, hi = (1-t)*lo + t*s, (1-t)*s + t*hi
    out = w @ values

Kernel strategy (data-parallel over 8 cores, 16384 points/core):
  * w^T resident in SBUF as [128 nodes, 16384 points] in bf16; points processed
    in 32 chunks of 512, two chunks paired per PSUM tile ([128, 1024] = 2 banks)
    so DVE/ACT fixed costs amortize.
  * All matmul operands are bf16: weight loads use FWL and the PE streams
    multiple columns/cycle (~60ns per [128x128]@[128x512] matmul measured, vs
    ~390ns for fp32r with rotating weights). fp32 PSUM accumulation keeps the
    numerics at rel err ~8e-3 (gate is 2e-2).
  * Parameter transforms (sigmoid(4p-2), 3v+1 tile, one-hot placed splitE and
    esel broadcast masks, pushed-through value vectors) are tiny and
    precomputed on host in make_in_maps.
  * Per depth, per pair of chunks:
      - tb2 = broadcast matmuls (lhsT = one-hot esel rows) replicate the two
        chunks' t rows across 128 partitions into one 2-bank PSUM tile.
      - vv2 = w_pair * tb2 (one DVE op over [128, 1024]).
      - wn2 = L^T w_chunk + (R-L)^T vv_chunk per half (4 accumulating matmuls).
      - one ACT copy moves wn2 PSUM -> bf16 SBUF.
  * sdot matvec: 16 accumulating matmuls per phase with one-hot-placed split
    columns stack 16 chunks into one [16, 512] PSUM tile (lagged behind the
    copies).
  * Final depth is fused into the output matvec: y = (L@values)^T w_9 +
    ((R-L)@values)^T v_9, so depth 9 emits no w-update matmuls and no copy.
  * Row math runs 16-chunks-wide on partitions 0..15 with the substitution
    xml = x - lo, d = hi - lo:
        g = sdot * d; t = sigmoid(10*(xml - g)); xml' = xml - t*g
        d' = g - 2*t*g + t*d
    Phase k runs group k%2 at depth k//2; row math for phase k+1 is emitted
    before phase k's pairs so its DVE->ACT chain hides under phase k's work.
"""

from contextlib import ExitStack

import ml_dtypes
import numpy as np

import concourse.bacc as bacc
import concourse.tile as tile
from concourse import mybir
from concourse.bass_utils import run_bass_kernel_spmd

F32 = mybir.dt.float32
BF16 = mybir.dt.bfloat16
NP_BF16 = ml_dtypes.bfloat16
AOP = mybir.AluOpType
AFT = mybir.ActivationFunctionType

N_TOTAL = 131072
NCORES = 8
NPTS = N_TOTAL // NCORES      # 16384 points per core
F = 512                       # points per chunk (one PSUM bank of fp32)
M = 128                       # fractal nodes
NCH = 32                      # chunks (= partitions used for batched row math)
DEPTH = 10
INV_SMOOTH = 10.0             # 1 / smoothing_width
G = NCH // 2                  # 16 chunks per phase group
MV_LAG = 4                    # chunks of lag before issuing sdot matvecs


def _emit(nc, bench_reps=1):
    x_in = nc.declare_dram_parameter("x", [NPTS], F32, isOutput=False)
    l16_in = nc.declare_dram_parameter("l16", [M, M], BF16, isOutput=False)
    rml16_in = nc.declare_dram_parameter("rml16", [M, M], BF16, isOutput=False)
    splitE_in = nc.declare_dram_parameter("splitE", [M, G * G], BF16, isOutput=False)
    lvE_in = nc.declare_dram_parameter("lvE", [M, G * G], BF16, isOutput=False)
    rvE_in = nc.declare_dram_parameter("rvE", [M, G * G], BF16, isOutput=False)
    esel_in = nc.declare_dram_parameter("esel", [G, G * M], BF16, isOutput=False)
    l0col_in = nc.declare_dram_parameter("l0col", [M, 1], F32, isOutput=False)
    rml0_in = nc.declare_dram_parameter("rml0", [M, 1], F32, isOutput=False)
    sp0_in = nc.declare_dram_parameter("sp0", [G, 1], F32, isOutput=False)
    b0_in = nc.declare_dram_parameter("b0", [G, 1], F32, isOutput=False)
    y_out = nc.declare_dram_parameter("y", [NPTS], F32, isOutput=True)

    with tile.TileContext(nc) as tc, ExitStack() as ctx:
        sing = ctx.enter_context(tc.tile_pool(name="sing", bufs=1))
        scratch = ctx.enter_context(tc.tile_pool(name="scratch", bufs=2))
        tpool = ctx.enter_context(tc.tile_pool(name="tpool", bufs=4))
        vpool = ctx.enter_context(tc.tile_pool(name="vpool", bufs=3))
        ps_t = ctx.enter_context(tc.tile_pool(name="ps_t", bufs=2, space="PSUM"))
        ps_w = ctx.enter_context(tc.tile_pool(name="ps_w", bufs=2, space="PSUM"))
        ps_sdot = ctx.enter_context(tc.tile_pool(name="ps_sdot", bufs=2, space="PSUM"))

        # ---- constants (host-precomputed, DMA'd once) ----
        def load(name, shape, dt, src):
            t = sing.tile(shape, dt, tag=name)
            nc.sync.dma_start(out=t, in_=src)
            return t

        l16 = load("l16", [M, M], BF16, l16_in[:, :])
        rml16 = load("rml16", [M, M], BF16, rml16_in[:, :])
        splitE = load("splitE", [M, G * G], BF16, splitE_in[:, :])
        lvE = load("lvE", [M, G * G], BF16, lvE_in[:, :])
        rvE = load("rvE", [M, G * G], BF16, rvE_in[:, :])
        esel = load("esel", [G, G * M], BF16, esel_in[:, :])
        l0col = load("l0col", [M, 1], F32, l0col_in[:, :])
        rml0 = load("rml0", [M, 1], F32, rml0_in[:, :])
        sp0 = load("sp0", [G, 1], F32, sp0_in[:, :])
        b0 = load("b0", [G, 1], F32, b0_in[:, :])

        def esel_slice(i):
            return esel[:, i * M : (i + 1) * M]

        # ---- resident state (per group) ----
        w_bufs = [
            sing.tile([M, NPTS], BF16, tag="w_ping", name="w_ping"),
            sing.tile([M, NPTS], BF16, tag="w_pong", name="w_pong"),
        ]
        half = G * F
        xml = [
            sing.tile([G, F], F32, tag="xml0", name="xml0"),
            sing.tile([G, F], F32, tag="xml1", name="xml1"),
        ]
        dd = [
            sing.tile([G, F], F32, tag="dd0", name="dd0"),
            sing.tile([G, F], F32, tag="dd1", name="dd1"),
        ]

        def body():
         for g in range(2):
            nc.sync.dma_start(
                out=xml[g],
                in_=x_in[g * half : (g + 1) * half].rearrange("(p f) -> p f", f=F),
            )

         # ---- depth 0: closed form state; w1 via fused outer-product blend ----
         ttile = [None, None]
         for g in range(2):
             t0 = tpool.tile([G, F], BF16, tag="t", name=f"t0g{g}")
             nc.scalar.activation(t0, xml[g], AFT.Sigmoid, bias=b0, scale=INV_SMOOTH)
             tgt = scratch.tile([G, F], F32, tag="tgt")
             nc.gpsimd.tensor_scalar_mul(tgt, t0, sp0)
             nc.gpsimd.tensor_sub(xml[g], xml[g], tgt)
             ee = scratch.tile([G, F], F32, tag="ee")
             nc.gpsimd.tensor_scalar_mul(ee, tgt, -2.0)
             nc.gpsimd.tensor_add(ee, ee, t0)
             nc.gpsimd.tensor_scalar(dd[g], ee, sp0, None, op0=AOP.add)
             ttile[g] = t0

         # Phase schedule: phases k = 0..2*DEPTH-1 map to (depth, group).
         # Phase k runs group k%2 at depth k//2. The matvec feeding phase k+2
         # is accumulated (lagged) inside phase k's pair loop; the row math
         # for phase k+1 is emitted before phase k's pairs so its short
         # DVE->ACT chain hides under phase k's PE work.
         sdot_ps = [None] * (2 * DEPTH + 2)

         def row_math(k):
             """State update for phase k (depth k//2 >= 1, group k%2)."""
             g = k % 2
             sdot = sdot_ps[k]
             gt = scratch.tile([G, F], F32, tag="gt", name=f"gt{k}")
             nc.vector.tensor_mul(gt, sdot, dd[g])
             xms = scratch.tile([G, F], F32, tag="xms", name=f"xms{k}")
             nc.vector.tensor_sub(xms, xml[g], gt)
             tg_t = tpool.tile([G, F], BF16, tag="t", name=f"t{k}")
             nc.scalar.activation(tg_t, xms, AFT.Sigmoid, scale=INV_SMOOTH)
             if k // 2 < DEPTH - 1:
                 # xml/dd not needed after the last depth's t
                 tgt = scratch.tile([G, F], F32, tag="tgt", name=f"tgt{k}")
                 nc.gpsimd.tensor_mul(tgt, tg_t, gt)
                 nc.gpsimd.tensor_sub(xml[g], xml[g], tgt)
                 ee = scratch.tile([G, F], F32, tag="ee", name=f"ee{k}")
                 nc.gpsimd.tensor_scalar_mul(ee, tgt, -2.0)
                 nc.gpsimd.tensor_add(ee, ee, gt)
                 td = scratch.tile([G, F], F32, tag="td", name=f"td{k}")
                 nc.gpsimd.tensor_mul(td, tg_t, dd[g])
                 nc.gpsimd.tensor_add(dd[g], td, ee)
             ttile[g] = tg_t

         for k in range(2 * DEPTH):
             dep, g = k // 2, k % 2
             w_cur = w_bufs[dep % 2]
             w_new = w_bufs[(dep + 1) % 2]
             if 2 <= k + 1 < 2 * DEPTH:
                 row_math(k + 1)
             tg_t = ttile[g]
             sdot_next = ps_sdot.tile([G, F], F32, tag="sdot", name=f"sdot{k + 2}")
             sdot_ps[k + 2] = sdot_next
             final = dep == DEPTH - 1

             mv_q = []

             def flush_mv(limit):
                 while len(mv_q) > limit:
                     j = mv_q.pop(0)
                     cj = g * G + j
                     nc.tensor.matmul(
                         sdot_next,
                         lhsT=splitE[:, j * G : (j + 1) * G],
                         rhs=w_new[:, cj * F : (cj + 1) * F],
                         start=(j == 0),
                         stop=(j == G - 1),
                     )

             def make_tb(p):
                 """Broadcast pair p's t rows across partitions into PSUM."""
                 tb2 = ps_t.tile([M, 2 * F], F32, tag="tb2", name=f"tb2_{k}_{p}")
                 nc.tensor.matmul(
                     tb2[:, :F], lhsT=esel_slice(2 * p), rhs=tg_t, start=True, stop=True
                 )
                 nc.tensor.matmul(
                     tb2[:, F:], lhsT=esel_slice(2 * p + 1), rhs=tg_t,
                     start=True, stop=True,
                 )
                 return tb2

             # PE stream is in-order: emit pair p+1's broadcast before pair p's
             # vv-dependent matmuls so the PE never idles behind the DVE.
             tb_next = make_tb(0)
             for p in range(G // 2):
                 c0 = g * G + 2 * p
                 tb2 = tb_next
                 if p + 1 < G // 2:
                     tb_next = make_tb(p + 1)
                 flush_mv(MV_LAG)
                 if dep == 0:
                     # w1 = t_b * (R-L)[0,:] + L[0,:], straight to SBUF bf16
                     nc.vector.tensor_scalar(
                         w_new[:, c0 * F : (c0 + 2) * F], tb2, rml0, l0col,
                         op0=AOP.mult, op1=AOP.add,
                     )
                 else:
                     vv2 = vpool.tile([M, 2 * F], BF16, tag="vv2")
                     nc.vector.tensor_mul(vv2, w_cur[:, c0 * F : (c0 + 2) * F], tb2)
                     if final:
                         # y += (L@vals)^T w_9 + ((R-L)@vals)^T v_9 per chunk
                         for q in range(2):
                             j = 2 * p + q
                             cj = c0 + q
                             nc.tensor.matmul(
                                 sdot_next,
                                 lhsT=lvE[:, j * G : (j + 1) * G],
                                 rhs=w_cur[:, cj * F : (cj + 1) * F],
                                 start=(j == 0),
                                 stop=False,
                             )
                             nc.tensor.matmul(
                                 sdot_next,
                                 lhsT=rvE[:, j * G : (j + 1) * G],
                                 rhs=vv2[:, q * F : (q + 1) * F],
                                 start=False,
                                 stop=(j == G - 1),
                             )
                         continue
                     wns = []
                     for q in range(2):
                         cj = c0 + q
                         wn = ps_w.tile([M, F], F32, tag="wn", name=f"wn{k}_{cj}")
                         nc.tensor.matmul(
                             wn, lhsT=l16,
                             rhs=w_cur[:, cj * F : (cj + 1) * F],
                             start=True, stop=False,
                         )
                         wns.append(wn)
                     for q in range(2):
                         nc.tensor.matmul(
                             wns[q], lhsT=rml16, rhs=vv2[:, q * F : (q + 1) * F],
                             start=False, stop=True,
                         )
                     for q in range(2):
                         cj = c0 + q
                         nc.scalar.copy(w_new[:, cj * F : (cj + 1) * F], wns[q])
                 if not final:
                     mv_q.append(2 * p)
                     mv_q.append(2 * p + 1)
             flush_mv(0)

         # ---- output: y accumulated per group in the final sdot tiles ----
         for g in range(2):
             ysb = scratch.tile([G, F], F32, tag="ysb", bufs=1, name=f"ysb{g}")
             nc.scalar.copy(ysb, sdot_ps[2 * DEPTH + g])
             nc.sync.dma_start(
                 out=y_out[g * half : (g + 1) * half].rearrange("(p f) -> p f", f=F),
                 in_=ysb,
             )

        if bench_reps > 1:
            with tc.For_i(0, bench_reps, 1):
                body()
        else:
            body()

    return nc


_CACHE = {}


def build_bench(reps):
    """Fresh module with the whole computation repeated `reps` times on-device."""
    nc = bacc.Bacc("TRN2", target_bir_lowering=False)
    _emit(nc, bench_reps=reps)
    nc.compile()
    return nc


def build_bass(compiled=True):
    """Build (and by default finalize) the Bacc module.

    compiled=False returns the pre-compile module for CoreSim runs.
    """
    if "nc" not in _CACHE:
        nc = bacc.Bacc("TRN2", target_bir_lowering=False)
        _emit(nc)
        _CACHE["nc"] = nc
    nc = _CACHE["nc"]
    if compiled and not _CACHE.get("compiled"):
        nc.compile()
        _CACHE["compiled"] = True
    return nc


def make_in_maps(x, split_points_param, values_param, left_matrix, right_matrix):
    x = np.ascontiguousarray(x, dtype=np.float32)
    shards = x.reshape(NCORES, NPTS)

    spp = np.asarray(split_points_param, dtype=np.float32)
    vp = np.asarray(values_param, dtype=np.float32)
    L = np.asarray(left_matrix, dtype=np.float32)
    R = np.asarray(right_matrix, dtype=np.float32)

    split = (1.0 / (1.0 + np.exp(-(4.0 * spp - 2.0)))).astype(np.float32)
    values = np.tile(vp * 3.0 + 1.0, M // vp.shape[0]).astype(np.float32)
    lv = L @ values
    rv = (R - L) @ values

    splitE = np.zeros((M, G * G), NP_BF16)
    lvE = np.zeros((M, G * G), NP_BF16)
    rvE = np.zeros((M, G * G), NP_BF16)
    for i in range(G):
        splitE[:, i * G + i] = split
        lvE[:, i * G + i] = lv
        rvE[:, i * G + i] = rv
    esel = np.zeros((G, G * M), NP_BF16)
    for i in range(G):
        esel[i, i * M : (i + 1) * M] = 1.0

    l0col = L[0, :].reshape(M, 1).astype(np.float32)
    rml0 = (R[0, :] - L[0, :]).reshape(M, 1).astype(np.float32)
    sp0 = np.full((G, 1), split[0], np.float32)
    b0 = np.full((G, 1), -INV_SMOOTH * split[0], np.float32)

    common = {
        "l16": L.astype(NP_BF16),
        "rml16": (R - L).astype(NP_BF16),
        "splitE": splitE,
        "lvE": lvE,
        "rvE": rvE,
        "esel": esel,
        "l0col": l0col,
        "rml0": rml0,
        "sp0": sp0,
        "b0": b0,
    }
    return [{"x": shards[i], **common} for i in range(NCORES)]


def kernel(x, split_points_param, values_param, left_matrix, right_matrix, max_depth):
    assert int(max_depth) == DEPTH
    nc = build_bass()
    in_maps = make_in_maps(
        x, split_points_param, values_param, left_matrix, right_matrix
    )
    res = run_bass_kernel_spmd(nc, in_maps, list(range(NCORES)))
    out = np.concatenate([res.results[i]["y"] for i in range(NCORES)])
    return out.astype(np.float32)


# revision 17
# speedup vs baseline: 1.6273x; 1.5125x over previous
"""Trainium2 Bass kernel for nn_Fractal1D (soft fractal / smoothed decision-tree descent).

Reference computation (per point x, N=131072 points, M=128 nodes, depth 10):
    split = sigmoid(4*p - 2); values = tile(3*v + 1, 4)
    w0 = e_0;  lo=0, hi=1
    repeat 10x:
        s  = lo + (w @ split) * (hi - lo)
        t  = sigmoid((x - s) / 0.1)
        w  = (1-t) * (w @ L) + t * (w @ R)
        lo, hi = (1-t)*lo + t*s, (1-t)*s + t*hi
    out = w @ values

Kernel strategy (data-parallel over 8 cores, 16384 points/core):
  * w^T resident in SBUF as [128 nodes, 16384 points] in bf16; points processed
    in 32 chunks of 512, two chunks paired per PSUM tile ([128, 1024] = 2 banks)
    so DVE/ACT fixed costs amortize.
  * All matmul operands are bf16: weight loads use FWL and the PE streams
    multiple columns/cycle (~60ns per [128x128]@[128x512] matmul measured, vs
    ~390ns for fp32r with rotating weights). fp32 PSUM accumulation keeps the
    numerics at rel err ~8e-3 (gate is 2e-2).
  * Parameter transforms (sigmoid(4p-2), 3v+1 tile, one-hot placed splitE and
    esel broadcast masks, pushed-through value vectors) are tiny and
    precomputed on host in make_in_maps.
  * Per depth, per pair of chunks:
      - tb2 = broadcast matmuls (lhsT = one-hot esel rows) replicate the two
        chunks' t rows across 128 partitions into one 2-bank PSUM tile.
      - vv2 = w_pair * tb2 (one DVE op over [128, 1024]).
      - wn2 = L^T w_chunk + (R-L)^T vv_chunk per half (4 accumulating matmuls).
      - one ACT copy moves wn2 PSUM -> bf16 SBUF.
  * sdot matvec: 16 accumulating matmuls per phase with one-hot-placed split
    columns stack 16 chunks into one [16, 512] PSUM tile (lagged behind the
    copies).
  * Final depth is fused into the output matvec: y = (L@values)^T w_9 +
    ((R-L)@values)^T v_9, so depth 9 emits no w-update matmuls and no copy.
  * Row math runs 16-chunks-wide on partitions 0..15 with the substitution
    xml = x - lo, d = hi - lo:
        g = sdot * d; t = sigmoid(10*(xml - g)); xml' = xml - t*g
        d' = g - 2*t*g + t*d
    Phase k runs group k%2 at depth k//2; row math for phase k+1 is emitted
    before phase k's pairs so its DVE->ACT chain hides under phase k's work.
"""

from contextlib import ExitStack

import ml_dtypes
import numpy as np

import concourse.bacc as bacc
import concourse.tile as tile
from concourse import mybir
from concourse.bass_utils import run_bass_kernel_spmd

F32 = mybir.dt.float32
BF16 = mybir.dt.bfloat16
NP_BF16 = ml_dtypes.bfloat16
AOP = mybir.AluOpType
AFT = mybir.ActivationFunctionType

N_TOTAL = 131072
NCORES = 8
NPTS = N_TOTAL // NCORES      # 16384 points per core
F = 512                       # points per chunk (one PSUM bank of fp32)
M = 128                       # fractal nodes
NCH = 32                      # chunks (= partitions used for batched row math)
DEPTH = 10
INV_SMOOTH = 10.0             # 1 / smoothing_width
G = NCH // 2                  # 16 chunks per phase group
MV_LAG = 4                    # chunks of lag before issuing sdot matvecs


def _emit(nc, bench_reps=1):
    x_in = nc.declare_dram_parameter("x", [NPTS], F32, isOutput=False)
    l16_in = nc.declare_dram_parameter("l16", [M, M], BF16, isOutput=False)
    rml16_in = nc.declare_dram_parameter("rml16", [M, M], BF16, isOutput=False)
    splitE_in = nc.declare_dram_parameter("splitE", [M, G * G], BF16, isOutput=False)
    yE_in = nc.declare_dram_parameter("yE", [M, G * M], BF16, isOutput=False)
    esel_in = nc.declare_dram_parameter("esel", [G, G * M], BF16, isOutput=False)
    l0col_in = nc.declare_dram_parameter("l0col", [M, 1], F32, isOutput=False)
    rml0_in = nc.declare_dram_parameter("rml0", [M, 1], F32, isOutput=False)
    sp0_in = nc.declare_dram_parameter("sp0", [G, 1], F32, isOutput=False)
    b0_in = nc.declare_dram_parameter("b0", [G, 1], F32, isOutput=False)
    y_out = nc.declare_dram_parameter("y", [NPTS], F32, isOutput=True)

    with tile.TileContext(nc) as tc, ExitStack() as ctx:
        sing = ctx.enter_context(tc.tile_pool(name="sing", bufs=1))
        scratch = ctx.enter_context(tc.tile_pool(name="scratch", bufs=2))
        tpool = ctx.enter_context(tc.tile_pool(name="tpool", bufs=4))
        vpool = ctx.enter_context(tc.tile_pool(name="vpool", bufs=3))
        ps_t = ctx.enter_context(tc.tile_pool(name="ps_t", bufs=3, space="PSUM"))
        ps_w = ctx.enter_context(tc.tile_pool(name="ps_w", bufs=2, space="PSUM"))
        ps_sdot = ctx.enter_context(tc.tile_pool(name="ps_sdot", bufs=1, space="PSUM"))

        # ---- constants (host-precomputed, DMA'd once) ----
        def load(name, shape, dt, src):
            t = sing.tile(shape, dt, tag=name)
            nc.sync.dma_start(out=t, in_=src)
            return t

        l16 = load("l16", [M, M], BF16, l16_in[:, :])
        rml16 = load("rml16", [M, M], BF16, rml16_in[:, :])
        splitE = load("splitE", [M, G * G], BF16, splitE_in[:, :])
        yE = load("yE", [M, G * M], BF16, yE_in[:, :])
        esel = load("esel", [G, G * M], BF16, esel_in[:, :])
        l0col = load("l0col", [M, 1], F32, l0col_in[:, :])
        rml0 = load("rml0", [M, 1], F32, rml0_in[:, :])
        sp0 = load("sp0", [G, 1], F32, sp0_in[:, :])
        b0 = load("b0", [G, 1], F32, b0_in[:, :])

        def esel_slice(i):
            return esel[:, i * M : (i + 1) * M]

        # ---- resident state (per group) ----
        w_bufs = [
            sing.tile([M, NPTS], BF16, tag="w_ping", name="w_ping"),
            sing.tile([M, NPTS], BF16, tag="w_pong", name="w_pong"),
        ]
        half = G * F
        xml = [
            sing.tile([G, F], F32, tag="xml0", name="xml0"),
            sing.tile([G, F], F32, tag="xml1", name="xml1"),
        ]
        dd = [
            sing.tile([G, F], F32, tag="dd0", name="dd0"),
            sing.tile([G, F], F32, tag="dd1", name="dd1"),
        ]

        def body():
         for g in range(2):
            nc.sync.dma_start(
                out=xml[g],
                in_=x_in[g * half : (g + 1) * half].rearrange("(p f) -> p f", f=F),
            )

         # ---- depth 0: closed form state; w1 via fused outer-product blend ----
         ttile = [None, None]
         for g in range(2):
             t0 = tpool.tile([G, F], BF16, tag="t", name=f"t0g{g}")
             nc.scalar.activation(t0, xml[g], AFT.Sigmoid, bias=b0, scale=INV_SMOOTH)
             tgt = scratch.tile([G, F], F32, tag="tgt")
             nc.gpsimd.tensor_scalar_mul(tgt, t0, sp0)
             nc.gpsimd.tensor_sub(xml[g], xml[g], tgt)
             ee = scratch.tile([G, F], F32, tag="ee")
             nc.gpsimd.tensor_scalar_mul(ee, tgt, -2.0)
             nc.gpsimd.tensor_add(ee, ee, t0)
             nc.gpsimd.tensor_scalar(dd[g], ee, sp0, None, op0=AOP.add)
             ttile[g] = t0

         # Phase schedule: phases k = 0..2*DEPTH-1 map to (depth, group).
         # Phase k runs group k%2 at depth k//2. The matvec feeding phase k+2
         # is accumulated (lagged) inside phase k's pair loop; the row math
         # for phase k+1 is emitted before phase k's pairs so its short
         # DVE->ACT chain hides under phase k's PE work.
         sdot_ps = [None] * (2 * DEPTH + 2)

         def row_math(k):
             """State update for phase k (depth k//2 >= 1, group k%2)."""
             g = k % 2
             sdot = sdot_ps[k]
             gt = scratch.tile([G, F], F32, tag="gt", name=f"gt{k}")
             nc.vector.tensor_mul(gt, sdot[0:G, :], dd[g])
             xms = scratch.tile([G, F], F32, tag="xms", name=f"xms{k}")
             nc.gpsimd.tensor_sub(xms, xml[g], gt)
             tg_t = tpool.tile([G, F], BF16, tag="t", name=f"t{k}")
             nc.scalar.activation(tg_t, xms, AFT.Sigmoid, scale=INV_SMOOTH)
             if k // 2 < DEPTH - 1:
                 # xml/dd not needed after the last depth's t
                 tgt = scratch.tile([G, F], F32, tag="tgt", name=f"tgt{k}")
                 nc.gpsimd.tensor_mul(tgt, tg_t, gt)
                 nc.gpsimd.tensor_sub(xml[g], xml[g], tgt)
                 ee = scratch.tile([G, F], F32, tag="ee", name=f"ee{k}")
                 nc.gpsimd.tensor_scalar_mul(ee, tgt, -2.0)
                 nc.gpsimd.tensor_add(ee, ee, gt)
                 td = scratch.tile([G, F], F32, tag="td", name=f"td{k}")
                 nc.gpsimd.tensor_mul(td, tg_t, dd[g])
                 nc.gpsimd.tensor_add(dd[g], td, ee)
             ttile[g] = tg_t

         for k in range(2 * DEPTH):
             dep, g = k // 2, k % 2
             w_cur = w_bufs[dep % 2]
             w_new = w_bufs[(dep + 1) % 2]
             tg_t = ttile[g]
             final = dep == DEPTH - 1
             if not final:
                 sdot_next = ps_sdot.tile([M, F], F32, tag="sdot", name=f"sdot{k + 2}")
                 sdot_ps[k + 2] = sdot_next

             mv_q = []

             def flush_mv(limit):
                 while len(mv_q) > limit:
                     j = mv_q.pop(0)
                     cj = g * G + j
                     nc.tensor.matmul(
                         sdot_next[0:G, :],
                         lhsT=splitE[:, j * G : (j + 1) * G],
                         rhs=w_new[:, cj * F : (cj + 1) * F],
                         start=(j == 0),
                         stop=(j == G - 1),
                     )

             # Software-pipelined chunk loop: stage 1 (tb broadcast, L matmul,
             # vv) runs two chunks ahead of stage 2 (RmL matmul, copy, lagged
             # matvec) so the in-order PE stream never makes the DVE wait for
             # a cross-engine round trip.
             if final:
                 # t factors out of the value dots: y = (L@v)^T w_9 +
                 # t_9 * ((R-L)@v)^T w_9. One matmul per chunk accumulates
                 # both dot sets into a single bank (A rows 0..15, B rows
                 # 32..47 via one-hot column placement in yE).
                 yab = ps_sdot.tile([M, F], F32, tag="sdot", name=f"yab{g}")
                 for c in range(G):
                     ci = g * G + c
                     nc.tensor.matmul(
                         yab,
                         lhsT=yE[:, c * M : (c + 1) * M],
                         rhs=w_cur[:, ci * F : (ci + 1) * F],
                         start=(c == 0),
                         stop=(c == G - 1),
                     )
                     if c == 2 and k + 1 < 2 * DEPTH:
                         row_math(k + 1)
                 tg9 = ttile[g]
                 ym = scratch.tile([G, F], F32, tag="ym", name=f"ym{g}")
                 nc.vector.tensor_mul(ym, tg9, yab[32 : 32 + G, :])
                 ysb = scratch.tile([G, F], F32, tag="ysb", name=f"ysb{g}")
                 nc.vector.tensor_add(ysb, ym, yab[0:G, :])
                 nc.sync.dma_start(
                     out=y_out[g * half : (g + 1) * half].rearrange(
                         "(p f) -> p f", f=F
                     ),
                     in_=ysb,
                 )
                 continue

             tb_t = [None] * G
             vv_t = [None] * G
             wn_t = [None] * (G // 2)
             for c in range(G + 2):
                 if c < G:
                     ci = g * G + c
                     wsl = w_cur[:, ci * F : (ci + 1) * F]
                     tb = ps_t.tile([M, F], F32, tag="tb", name=f"tb{k}_{c}")
                     nc.tensor.matmul(
                         tb, lhsT=esel_slice(c), rhs=tg_t, start=True, stop=True
                     )
                     tb_t[c] = tb
                     if dep == 0:
                         # w1 = t_b*(R-L)[0,:] + L[0,:]; w1 >= 0 so the ACT
                         # Relu form is exact. Alternate engines to balance.
                         if c % 2 == 0:
                             nc.vector.tensor_scalar(
                                 w_new[:, ci * F : (ci + 1) * F], tb, rml0, l0col,
                                 op0=AOP.mult, op1=AOP.add,
                             )
                         else:
                             nc.scalar.activation(
                                 w_new[:, ci * F : (ci + 1) * F], tb, AFT.Relu,
                                 bias=l0col, scale=rml0,
                             )
                         mv_q.append(c)
                         flush_mv(MV_LAG)
                     else:
                         if c % 2 == 0:
                             wn_t[c // 2] = ps_w.tile(
                                 [M, 2 * F], F32, tag="wn2", name=f"wn2_{k}_{c // 2}"
                             )
                         nc.tensor.matmul(
                             wn_t[c // 2][:, (c % 2) * F : (c % 2 + 1) * F],
                             lhsT=l16, rhs=wsl, start=True, stop=False,
                         )
                         vv = vpool.tile([M, F], BF16, tag="vv", name=f"vv{k}_{c}")
                         nc.vector.tensor_mul(vv, wsl, tb)
                         vv_t[c] = vv
                 if c == 2 and 2 <= k + 1 < 2 * DEPTH:
                     row_math(k + 1)
                 d = c - 2
                 if 0 <= d < G and dep >= 1:
                     nc.tensor.matmul(
                         wn_t[d // 2][:, (d % 2) * F : (d % 2 + 1) * F],
                         lhsT=rml16, rhs=vv_t[d], start=False, stop=True,
                     )
                     if d % 2 == 1:
                         pr = d // 2
                         c0 = g * G + 2 * pr
                         nc.scalar.copy(
                             w_new[:, c0 * F : (c0 + 2) * F], wn_t[pr]
                         )
                         mv_q.append(2 * pr)
                         mv_q.append(2 * pr + 1)
                         flush_mv(MV_LAG)
             flush_mv(0)

        if bench_reps > 1:
            with tc.For_i(0, bench_reps, 1):
                body()
        else:
            body()

    return nc


_CACHE = {}


def build_bench(reps):
    """Fresh module with the whole computation repeated `reps` times on-device."""
    nc = bacc.Bacc("TRN2", target_bir_lowering=False)
    _emit(nc, bench_reps=reps)
    nc.compile()
    return nc


def build_bass(compiled=True):
    """Build (and by default finalize) the Bacc module.

    compiled=False returns the pre-compile module for CoreSim runs.
    """
    if "nc" not in _CACHE:
        nc = bacc.Bacc("TRN2", target_bir_lowering=False)
        _emit(nc)
        _CACHE["nc"] = nc
    nc = _CACHE["nc"]
    if compiled and not _CACHE.get("compiled"):
        nc.compile()
        _CACHE["compiled"] = True
    return nc


def make_in_maps(x, split_points_param, values_param, left_matrix, right_matrix):
    x = np.ascontiguousarray(x, dtype=np.float32)
    shards = x.reshape(NCORES, NPTS)

    spp = np.asarray(split_points_param, dtype=np.float32)
    vp = np.asarray(values_param, dtype=np.float32)
    L = np.asarray(left_matrix, dtype=np.float32)
    R = np.asarray(right_matrix, dtype=np.float32)

    split = (1.0 / (1.0 + np.exp(-(4.0 * spp - 2.0)))).astype(np.float32)
    values = np.tile(vp * 3.0 + 1.0, M // vp.shape[0]).astype(np.float32)
    lv = L @ values
    rv = (R - L) @ values

    splitE = np.zeros((M, G * G), NP_BF16)
    for i in range(G):
        splitE[:, i * G + i] = split
    yE = np.zeros((M, G * M), NP_BF16)
    for i in range(G):
        yE[:, i * M + i] = lv
        yE[:, i * M + 32 + i] = rv
    esel = np.zeros((G, G * M), NP_BF16)
    for i in range(G):
        esel[i, i * M : (i + 1) * M] = 1.0

    l0col = L[0, :].reshape(M, 1).astype(np.float32)
    rml0 = (R[0, :] - L[0, :]).reshape(M, 1).astype(np.float32)
    sp0 = np.full((G, 1), split[0], np.float32)
    b0 = np.full((G, 1), -INV_SMOOTH * split[0], np.float32)

    common = {
        "l16": L.astype(NP_BF16),
        "rml16": (R - L).astype(NP_BF16),
        "splitE": splitE,
        "yE": yE,
        "esel": esel,
        "l0col": l0col,
        "rml0": rml0,
        "sp0": sp0,
        "b0": b0,
    }
    return [{"x": shards[i], **common} for i in range(NCORES)]


def kernel(x, split_points_param, values_param, left_matrix, right_matrix, max_depth):
    assert int(max_depth) == DEPTH
    nc = build_bass()
    in_maps = make_in_maps(
        x, split_points_param, values_param, left_matrix, right_matrix
    )
    res = run_bass_kernel_spmd(nc, in_maps, list(range(NCORES)))
    out = np.concatenate([res.results[i]["y"] for i in range(NCORES)])
    return out.astype(np.float32)
